# revision 15
# baseline (speedup 1.0000x reference)
_last_device_wall_ns = None
"""Trainium2 Bass kernel for nn_KANOnlyTextModel (2-layer KAN text model).

Algorithm
---------
Layer 1's input x = emb[idx].reshape(B, S*D) takes values only from the 128
rows of emb, so the layer-1 spline+silu contraction factors through tiny
per-token-position lookup tables T_s[v, o] (computed on the host from emb,
coef1, sb1 — a batch-independent weight transform), giving
    y1[b, o] = sum_s T_s[idx[b, s], o].
All four affine stages are folded in on the host: subnode/node affine 1 into
T (T' = a1*T + c1/S), affine 2 into the layer-2 planes (scale by a2, plus a
constant plane carrying c2 against an all-ones feature).

Device work per core (SPMD over 8 cores, sharded over token positions s for
layer 1 and over the vocab dim for layer 2):
  * build a one-hot matrix from this core's 8 token positions of idx
    (iota + is_equal over a partition-broadcast row),
  * 16 fp16 one-hot matmuls -> partial y1[o, b] for the full batch,
  * fp16 AllReduce sums the partials (every core gets the full y1),
  * layer 2 for the full batch, this core's 16 vocab rows: cubic B-spline
    basis via the truncated-power identity (relu^3 planes band-combined in
    fp32), then 16 fp16 matmuls against this core's 8 weight planes
    (6 basis + silu + const), fp16 output [16, 1024].

Shipped per core: one packed fp16 tensor [128, 1152] (T slice + weight
planes) + the idx row ([1, 8192] fp16) — ~0.3 MB. The host reassembles
logits from the 8 vocab slices.
"""

import numpy as np


def _enable_jax_compile_cache():
    # The bass2jax axon path builds a fresh jit closure per dispatch, so the
    # in-memory jit cache never hits; the persistent cache turns the per-call
    # XLA+NEFF recompile (~250 ms) into a disk load.
    try:
        import jax
        if jax.config.jax_compilation_cache_dir is None:
            jax.config.update("jax_compilation_cache_dir",
                              "/tmp/.jax_bass_cache")
        jax.config.update("jax_persistent_cache_min_compile_time_secs", 0.0)
        jax.config.update("jax_persistent_cache_min_entry_size_bytes", 0)
    except Exception:
        pass


_enable_jax_compile_cache()

K = 3
NUM = 3
H_GRID = 2.0 / NUM
NK = NUM + K            # 6 basis fns
NJ = NUM + 2 * K + 1    # 10 knots
NP2 = NK + 2            # layer-2 planes: 6 basis + silu + const
GRID = (np.arange(-K, NUM + K + 1, dtype=np.float64) * H_GRID - 1.0)  # (10,)
BETA = (np.array([1, -4, 6, -4, 1], dtype=np.float64) / (6 * H_GRID ** 3))

B, S, V, D, H = 1024, 64, 128, 128, 128
N_CORES = 8
S_LOC = S // N_CORES    # 8 token positions per core
V_LOC = V // N_CORES    # 16 vocab rows per core (layer 2)
BCH = 512               # batch columns per PSUM chunk
TCOLS = S_LOC * H                 # 1024: T region of the packed input
PCOLS = TCOLS + NP2 * V_LOC       # 1152: + 8 weight planes of 16

_cached_nc = None


def _build_nc():
    import concourse.mybir as mybir
    import concourse.tile as tile
    from concourse import bacc

    f32 = mybir.dt.float32
    f16 = mybir.dt.float16
    AF = mybir.ActivationFunctionType
    ALU = mybir.AluOpType

    nc = bacc.Bacc("TRN2", target_bir_lowering=False, debug=False,
                   enable_asserts=False, num_devices=N_CORES)

    pk = nc.dram_tensor("pk", [128, PCOLS], f16, kind="ExternalInput")
    idx16 = nc.dram_tensor("idx16", [1, S_LOC * B], f16, kind="ExternalInput")
    out = nc.dram_tensor("out", [V_LOC, B], f16, kind="ExternalOutput")

    # split collective staging: one pair per batch half so the AllReduce of
    # half 0 overlaps the gather of half 1, and layer-2 compute of half 0
    # overlaps the AllReduce of half 1
    y1p_d = [nc.dram_tensor(f"y1p_d{i}", [H, BCH], f16) for i in range(B // BCH)]
    ar_out = [nc.dram_tensor(f"ar_out{i}", [H, BCH], f16) for i in range(B // BCH)]

    with tile.TileContext(nc) as tc:
        with (
            tc.tile_pool(name="big", bufs=1) as big,
            tc.tile_pool(name="tmp", bufs=2) as tmp,
            tc.tile_pool(name="ps_y", bufs=2, space="PSUM") as ps_y,
            tc.tile_pool(name="ps_l", bufs=2, space="PSUM") as ps_l,
        ):
            # ---- loads ----
            pk_sb = big.tile([128, PCOLS], f16, tag="pk_sb")
            nc.sync.dma_start(pk_sb[:], pk[:])
            idx_sb = big.tile([1, S_LOC * B], f16, tag="idx_sb")
            nc.sync.dma_start(idx_sb[:], idx16[:])

            # ---- one-hot from idx: oh[v, s*B+b] = (idx[b, s] == v) ----
            idxb = big.tile([128, S_LOC * B], f16, tag="idxb")
            nc.gpsimd.partition_broadcast(idxb[:], idx_sb[:])
            ic = big.tile([128, 1], f32, tag="ic")
            nc.gpsimd.iota(ic[:], pattern=[[0, 1]], base=0,
                           channel_multiplier=1,
                           allow_small_or_imprecise_dtypes=True)
            oh = big.tile([128, S_LOC * B], f16, tag="oh")
            nc.vector.tensor_scalar(oh[:], idxb[:], ic[:, 0:1], None,
                                    ALU.is_equal)

            # negg[p, j] = -GRID[j] = 3 - j*h, built from an iota row
            ij = big.tile([128, NJ], f32, tag="ij")
            nc.gpsimd.iota(ij[:], pattern=[[1, NJ]], base=0,
                           channel_multiplier=0,
                           allow_small_or_imprecise_dtypes=True)
            negg = big.tile([128, NJ], f32, tag="negg")
            nc.vector.tensor_scalar(negg[:], ij[:], float(-H_GRID), 3.0,
                                    ALU.mult, ALU.add)

            # ---- gather + AllReduce per batch half, pipelined: the AR of
            # half 0 is in flight while the gather of half 1 runs, and the
            # layer-2 compute of half 0 overlaps the AR of half 1 ----
            nh = B // BCH
            for hb in range(nh):
                yps = ps_y.tile([H, BCH], f32, tag="yps", name=f"yps{hb}")
                for s in range(S_LOC):
                    nc.tensor.matmul(
                        yps[:],
                        lhsT=pk_sb[:, s * H:(s + 1) * H],
                        rhs=oh[:, s * B + hb * BCH: s * B + (hb + 1) * BCH],
                        start=(s == 0), stop=(s == S_LOC - 1),
                    )
                y1h = big.tile([H, BCH], f16, tag=f"y1h{hb}")
                nc.vector.tensor_copy(y1h[:], yps[:])
                nc.sync.dma_start(y1p_d[hb][:], y1h[:])
                nc.gpsimd.collective_compute(
                    "AllReduce",
                    mybir.AluOpType.add,
                    replica_groups=[list(range(N_CORES))],
                    ins=[y1p_d[hb][:]],
                    outs=[ar_out[hb][:]],
                )

            # ---- layer 2 per batch half: relu^3 planes, band-combine,
            # 8 fp16 matmul planes for this core's 16 vocab rows ----
            lo = big.tile([V_LOC, B], f16, tag="lo")
            for hb in range(nh):
                yr = big.tile([H, BCH], f16, tag=f"yr{hb}")
                nc.sync.dma_start(yr[:], ar_out[hb][:])
                R = big.tile([128, NJ * BCH], f32, tag=f"R{hb}")
                for j in range(NJ):
                    r = tmp.tile([128, BCH], f32, tag="feat_r")
                    nc.scalar.activation(r[:], yr[:], AF.Relu,
                                         bias=negg[:, j:j + 1], scale=1.0)
                    rr = tmp.tile([128, BCH], f32, tag="feat_rr")
                    nc.vector.tensor_mul(rr[:], r[:], r[:])
                    nc.vector.tensor_mul(R[:, j * BCH:(j + 1) * BCH], rr[:], r[:])

                F6 = big.tile([128, NP2 * BCH], f16, tag=f"F6{hb}")
                for k in range(NK):
                    acc = tmp.tile([128, BCH], f32, tag="acc_a")
                    nc.vector.tensor_scalar(
                        acc[:], R[:, k * BCH:(k + 1) * BCH],
                        float(BETA[0]), None, ALU.mult)
                    for m in (1, 2, 3):
                        acc2 = tmp.tile([128, BCH], f32, tag="acc_b" if m % 2 else "acc_a")
                        nc.vector.scalar_tensor_tensor(
                            acc2[:], R[:, (k + m) * BCH:(k + m + 1) * BCH],
                            float(BETA[m]), acc[:], ALU.mult, ALU.add)
                        acc = acc2
                    nc.vector.scalar_tensor_tensor(
                        F6[:, k * BCH:(k + 1) * BCH],
                        R[:, (k + 4) * BCH:(k + 5) * BCH],
                        float(BETA[4]), acc[:], ALU.mult, ALU.add)
                nc.scalar.activation(F6[:, NK * BCH:(NK + 1) * BCH], yr[:], AF.Silu)
                nc.vector.memset(F6[:, (NK + 1) * BCH:NP2 * BCH], 1.0)

                log_ps = ps_l.tile([V_LOC, BCH], f32, tag="log_ps", name=f"lps{hb}")
                for k in range(NP2):
                    nc.tensor.matmul(
                        log_ps[:],
                        lhsT=pk_sb[:, TCOLS + k * V_LOC: TCOLS + (k + 1) * V_LOC],
                        rhs=F6[:, k * BCH:(k + 1) * BCH],
                        start=(k == 0), stop=(k == NP2 - 1),
                    )
                nc.vector.tensor_copy(lo[:, hb * BCH:(hb + 1) * BCH], log_ps[:])
            nc.sync.dma_start(out[:], lo[:])

    nc.compile()
    return nc


def _get_nc():
    global _cached_nc
    if _cached_nc is None:
        _cached_nc = _build_nc()
        # Warm the NEFF/XLA/axon caches so the first real dispatch is hot.
        from concourse.bass_utils import run_bass_kernel_spmd
        dummy = [{
            "pk": np.zeros((128, PCOLS), np.float16),
            "idx16": np.zeros((1, S_LOC * B), np.float16),
        } for _ in range(N_CORES)]
        try:
            run_bass_kernel_spmd(_cached_nc, dummy, core_ids=list(range(N_CORES)))
        except Exception:
            pass
    return _cached_nc


def _b_splines_host(x, grid):
    xe = x[..., None]
    g = np.broadcast_to(grid, x.shape + grid.shape)
    v = ((xe >= g[..., :-1]) & (xe < g[..., 1:])).astype(x.dtype)
    for j in range(1, K + 1):
        v = (xe - g[..., :-(j + 1)]) / (g[..., j:-1] - g[..., :-(j + 1)]) * v[..., :-1] \
          + (g[..., j + 1:] - xe) / (g[..., j + 1:] - g[..., 1:-j]) * v[..., 1:]
    return v


def _prepare_inputs(idx, emb, coef1, sb1, ss1, subs1, subb1, nodes1, nodeb1,
                    coef2, sb2, ss2, subs2, subb2, nodes2, nodeb2):
    idx = np.asarray(idx)
    emb64 = np.asarray(emb, np.float64)

    # T_s[v, o]: exact float64 basis on the tiny emb table, f32 contraction.
    basis = _b_splines_host(emb64, GRID)                     # (V, D, 6)
    silu = (emb64 / (1.0 + np.exp(-emb64))).astype(np.float32)
    ss1 = np.asarray(ss1, np.float32)
    ce1 = np.asarray(coef1, np.float32)
    if not np.all(ss1 == 1.0):
        ce1 = ce1 * ss1[:, :, None]                          # (S*D, H, 6)
    ce1 = np.ascontiguousarray(
        ce1.reshape(S, D, H, NK).transpose(0, 1, 3, 2)).reshape(S, D * NK, H)
    bf = np.ascontiguousarray(basis.reshape(V, D * NK).astype(np.float32))
    T = np.matmul(bf[None], ce1)                             # (S, V, H)
    T += np.matmul(silu[None], np.asarray(sb1, np.float32).reshape(S, D, H))

    # fold subnode/node affine 1 into T: h = a1*y1 + c1 = sum_s (a1*T + c1/S)
    a1 = (np.asarray(nodes1) * np.asarray(subs1)).astype(np.float32)
    c1 = (np.asarray(nodes1) * np.asarray(subb1) + np.asarray(nodeb1)).astype(np.float32)
    if not (np.all(a1 == 1.0) and np.all(c1 == 0.0)):
        T = a1[None, None, :] * T + (c1 / S)[None, None, :]
    T16 = T.astype(np.float16)

    # layer-2 planes scaled by affine 2: 6 basis (coef2*ss2*a2), silu (sb2*a2),
    # const (c2 against an all-ones feature, carried in partition row 0)
    a2 = (np.asarray(nodes2) * np.asarray(subs2)).astype(np.float32)
    c2 = (np.asarray(nodes2) * np.asarray(subb2) + np.asarray(nodeb2)).astype(np.float32)
    ss2 = np.asarray(ss2, np.float32)
    ce2 = np.asarray(coef2, np.float32)
    if not np.all(ss2 == 1.0):
        ce2 = ce2 * ss2[:, :, None]                          # (H, V, 6)
    ce2 = ce2 * a2[None, :, None]
    sb2a = np.asarray(sb2, np.float32) * a2[None, :]
    wconst = np.zeros((H, V), np.float32)
    wconst[0, :] = c2
    w2p_host = np.concatenate(
        [ce2, sb2a[:, :, None], wconst[:, :, None]], axis=2
    ).astype(np.float16)                                     # (H, V, 8)

    idxT = np.asarray(idx).T.astype(np.float16)              # (S, B)

    in_maps = []
    for c in range(N_CORES):
        sl = slice(c * S_LOC, (c + 1) * S_LOC)
        vsl = slice(c * V_LOC, (c + 1) * V_LOC)
        pk_core = np.empty((128, PCOLS), np.float16)
        pk_core[:, :TCOLS] = T16[sl].transpose(1, 0, 2).reshape(V, S_LOC * H)
        pk_core[:, TCOLS:] = w2p_host[:, vsl, :].transpose(0, 2, 1).reshape(H, NP2 * V_LOC)
        idx_core = np.ascontiguousarray(idxT[sl]).reshape(1, S_LOC * B)
        in_maps.append({"pk": pk_core, "idx16": idx_core})
    return in_maps


_last_results = None


def kernel(**inputs) -> np.ndarray:
    global _last_results, _last_device_wall_ns
    from concourse.bass_utils import run_bass_kernel_spmd
    import os

    nc = _get_nc()
    in_maps = _prepare_inputs(**inputs)
    trace = bool(int(os.environ.get("KAN_TRACE", "0")))
    import time as _t; _t0 = _t.perf_counter()
    res = run_bass_kernel_spmd(nc, in_maps, core_ids=list(range(N_CORES)),
                               trace=trace)
    _last_device_wall_ns = int((_t.perf_counter() - _t0) * 1e9)
    _last_results = res
    logits = np.concatenate(
        [res.results[c]["out"] for c in range(N_CORES)], axis=0)  # (V, B)
    return logits.T.astype(np.float32)


# revision 18
# speedup vs baseline: 1.2738x; 1.2738x over previous
_last_device_wall_ns = None
"""Trainium2 Bass kernel for nn_KANOnlyTextModel (2-layer KAN text model).

Algorithm
---------
Layer 1's input x = emb[idx].reshape(B, S*D) takes values only from the 128
rows of emb, so the layer-1 spline+silu contraction factors through tiny
per-token-position lookup tables T_s[v, o] (computed on the host from emb,
coef1, sb1 — a batch-independent weight transform), giving
    y1[b, o] = sum_s T_s[idx[b, s], o].
All four affine stages are folded in on the host: subnode/node affine 1 into
T (T' = a1*T + c1/S), affine 2 into the layer-2 planes (scale by a2, plus a
constant plane carrying c2 against an all-ones feature).

Device work per core (SPMD over 8 cores, sharded over token positions s for
layer 1 and over the vocab dim for layer 2):
  * build a one-hot matrix from this core's 8 token positions of idx
    (iota + is_equal over a partition-broadcast row),
  * 16 fp16 one-hot matmuls -> partial y1[o, b] for the full batch,
  * fp16 AllReduce sums the partials (every core gets the full y1),
  * layer 2 for the full batch, this core's 16 vocab rows: cubic B-spline
    basis via the truncated-power identity (relu^3 planes band-combined in
    fp32), then 16 fp16 matmuls against this core's 8 weight planes
    (6 basis + silu + const), fp16 output [16, 1024].

Shipped per core: one packed fp16 tensor [128, 1152] (T slice + weight
planes) + the idx row ([1, 8192] fp16) — ~0.3 MB. The host reassembles
logits from the 8 vocab slices.
"""

import numpy as np


def _enable_jax_compile_cache():
    # The bass2jax axon path builds a fresh jit closure per dispatch, so the
    # in-memory jit cache never hits; the persistent cache turns the per-call
    # XLA+NEFF recompile (~250 ms) into a disk load.
    try:
        import jax
        if jax.config.jax_compilation_cache_dir is None:
            jax.config.update("jax_compilation_cache_dir",
                              "/tmp/.jax_bass_cache")
        jax.config.update("jax_persistent_cache_min_compile_time_secs", 0.0)
        jax.config.update("jax_persistent_cache_min_entry_size_bytes", 0)
    except Exception:
        pass


_enable_jax_compile_cache()

K = 3
NUM = 3
H_GRID = 2.0 / NUM
NK = NUM + K            # 6 basis fns
NJ = NUM + 2 * K + 1    # 10 knots
NP2 = NK + 2            # layer-2 planes: 6 basis + silu + const
GRID = (np.arange(-K, NUM + K + 1, dtype=np.float64) * H_GRID - 1.0)  # (10,)
BETA = (np.array([1, -4, 6, -4, 1], dtype=np.float64) / (6 * H_GRID ** 3))

B, S, V, D, H = 1024, 64, 128, 128, 128
N_CORES = 8
S_LOC = S // N_CORES    # 8 token positions per core
V_LOC = V // N_CORES    # 16 vocab rows per core (layer 2)
BCH = 512               # batch columns per PSUM chunk
TCOLS = S_LOC * H                 # 1024: T region of the packed input
PCOLS = TCOLS + NP2 * V_LOC       # 1152: + 8 weight planes of 16

_cached_nc = None


def _build_nc():
    import concourse.mybir as mybir
    import concourse.tile as tile
    from concourse import bacc

    f32 = mybir.dt.float32
    f16 = mybir.dt.float16
    AF = mybir.ActivationFunctionType
    ALU = mybir.AluOpType

    nc = bacc.Bacc("TRN2", target_bir_lowering=False, debug=False,
                   enable_asserts=False, num_devices=N_CORES)

    pk = nc.dram_tensor("pk", [128, PCOLS], f16, kind="ExternalInput")
    idx16 = nc.dram_tensor("idx16", [1, S_LOC * B], f16, kind="ExternalInput")
    out = nc.dram_tensor("out", [V_LOC, B], f16, kind="ExternalOutput")

    y1p_d = nc.dram_tensor("y1p_d", [H, B], f16)
    ar_out = nc.dram_tensor("ar_out", [H, B], f16)

    with tile.TileContext(nc) as tc:
        with (
            tc.tile_pool(name="big", bufs=1) as big,
            tc.tile_pool(name="tmp", bufs=2) as tmp,
            tc.tile_pool(name="ps_y", bufs=2, space="PSUM") as ps_y,
            tc.tile_pool(name="ps_l", bufs=2, space="PSUM") as ps_l,
        ):
            # ---- loads ----
            pk_sb = big.tile([128, PCOLS], f16, tag="pk_sb")
            nc.sync.dma_start(pk_sb[:], pk[:])
            idx_sb = big.tile([1, S_LOC * B], f16, tag="idx_sb")
            nc.sync.dma_start(idx_sb[:], idx16[:])

            # ---- one-hot from idx: oh[v, s*B+b] = (idx[b, s] == v) ----
            idxb = big.tile([128, S_LOC * B], f16, tag="idxb")
            nc.gpsimd.partition_broadcast(idxb[:], idx_sb[:])
            ic = big.tile([128, 1], f32, tag="ic")
            nc.gpsimd.iota(ic[:], pattern=[[0, 1]], base=0,
                           channel_multiplier=1,
                           allow_small_or_imprecise_dtypes=True)
            oh = big.tile([128, S_LOC * B], f16, tag="oh")
            nc.vector.tensor_scalar(oh[:], idxb[:], ic[:, 0:1], None,
                                    ALU.is_equal)

            # ---- gather: partial y1[o, b] over this core's positions ----
            y1sb = big.tile([H, B], f16, tag="y1sb")
            for bc in range(B // BCH):
                yps = ps_y.tile([H, BCH], f32, tag="yps")
                for s in range(S_LOC):
                    nc.tensor.matmul(
                        yps[:],
                        lhsT=pk_sb[:, s * H:(s + 1) * H],
                        rhs=oh[:, s * B + bc * BCH: s * B + (bc + 1) * BCH],
                        start=(s == 0), stop=(s == S_LOC - 1),
                    )
                nc.vector.tensor_copy(y1sb[:, bc * BCH:(bc + 1) * BCH], yps[:])
            nc.sync.dma_start(y1p_d[:], y1sb[:])

            # ---- AllReduce: every core gets the full y1 (= h, affine folded) ----
            nc.gpsimd.collective_compute(
                "AllReduce",
                mybir.AluOpType.add,
                replica_groups=[list(range(N_CORES))],
                ins=[y1p_d[:]],
                outs=[ar_out[:]],
            )
            yr = big.tile([H, B], f16, tag="yr")
            nc.sync.dma_start(yr[:], ar_out[:])

            # ---- layer-2 features: relu^3 planes then band-combine ----
            # negg[p, j] = -GRID[j] = 3 - j*h, built from an iota row
            ij = big.tile([128, NJ], f32, tag="ij")
            nc.gpsimd.iota(ij[:], pattern=[[1, NJ]], base=0,
                           channel_multiplier=0,
                           allow_small_or_imprecise_dtypes=True)
            negg = big.tile([128, NJ], f32, tag="negg")
            nc.vector.tensor_scalar(negg[:], ij[:], float(-H_GRID), 3.0,
                                    ALU.mult, ALU.add)
            R = big.tile([128, NJ * B], f32, tag="R")
            for j in range(NJ):
                r = tmp.tile([128, B], f32, tag="feat_r")
                nc.scalar.activation(r[:], yr[:], AF.Relu,
                                     bias=negg[:, j:j + 1], scale=1.0)
                rr = tmp.tile([128, B], f32, tag="feat_rr")
                nc.vector.tensor_mul(rr[:], r[:], r[:])
                nc.vector.tensor_mul(R[:, j * B:(j + 1) * B], rr[:], r[:])

            F6 = big.tile([128, NP2 * B], f16, tag="F6")
            for k in range(NK):
                acc = tmp.tile([128, B], f32, tag="acc_a")
                nc.vector.tensor_scalar(
                    acc[:], R[:, k * B:(k + 1) * B],
                    float(BETA[0]), None, ALU.mult)
                for m in (1, 2, 3):
                    acc2 = tmp.tile([128, B], f32, tag="acc_b" if m % 2 else "acc_a")
                    nc.vector.scalar_tensor_tensor(
                        acc2[:], R[:, (k + m) * B:(k + m + 1) * B],
                        float(BETA[m]), acc[:], ALU.mult, ALU.add)
                    acc = acc2
                nc.vector.scalar_tensor_tensor(
                    F6[:, k * B:(k + 1) * B],
                    R[:, (k + 4) * B:(k + 5) * B],
                    float(BETA[4]), acc[:], ALU.mult, ALU.add)
            nc.scalar.activation(F6[:, NK * B:(NK + 1) * B], yr[:], AF.Silu)
            nc.vector.memset(F6[:, (NK + 1) * B:NP2 * B], 1.0)

            # ---- logits for this core's 16 vocab rows, full batch ----
            lo = big.tile([V_LOC, B], f16, tag="lo")
            for bc in range(B // BCH):
                log_ps = ps_l.tile([V_LOC, BCH], f32, tag="log_ps")
                for k in range(NP2):
                    nc.tensor.matmul(
                        log_ps[:],
                        lhsT=pk_sb[:, TCOLS + k * V_LOC: TCOLS + (k + 1) * V_LOC],
                        rhs=F6[:, k * B + bc * BCH: k * B + (bc + 1) * BCH],
                        start=(k == 0), stop=(k == NP2 - 1),
                    )
                nc.vector.tensor_copy(lo[:, bc * BCH:(bc + 1) * BCH], log_ps[:])
            nc.sync.dma_start(out[:], lo[:])

    nc.compile()
    return nc


def _get_nc():
    global _cached_nc
    if _cached_nc is None:
        _cached_nc = _build_nc()
        # Warm the NEFF/XLA/axon caches so the first real dispatch is hot.
        from concourse.bass_utils import run_bass_kernel_spmd
        dummy = [{
            "pk": np.zeros((128, PCOLS), np.float16),
            "idx16": np.zeros((1, S_LOC * B), np.float16),
        } for _ in range(N_CORES)]
        try:
            run_bass_kernel_spmd(_cached_nc, dummy, core_ids=list(range(N_CORES)))
        except Exception:
            pass
    return _cached_nc


def _b_splines_host(x, grid):
    xe = x[..., None]
    g = np.broadcast_to(grid, x.shape + grid.shape)
    v = ((xe >= g[..., :-1]) & (xe < g[..., 1:])).astype(x.dtype)
    for j in range(1, K + 1):
        v = (xe - g[..., :-(j + 1)]) / (g[..., j:-1] - g[..., :-(j + 1)]) * v[..., :-1] \
          + (g[..., j + 1:] - xe) / (g[..., j + 1:] - g[..., 1:-j]) * v[..., 1:]
    return v


def _prepare_inputs(idx, emb, coef1, sb1, ss1, subs1, subb1, nodes1, nodeb1,
                    coef2, sb2, ss2, subs2, subb2, nodes2, nodeb2):
    idx = np.asarray(idx)
    emb64 = np.asarray(emb, np.float64)

    # T_s[v, o]: exact float64 basis on the tiny emb table, f32 contraction.
    basis = _b_splines_host(emb64, GRID)                     # (V, D, 6)
    silu = (emb64 / (1.0 + np.exp(-emb64))).astype(np.float32)
    ss1 = np.asarray(ss1, np.float32)
    ce1 = np.asarray(coef1, np.float32)
    if not np.all(ss1 == 1.0):
        ce1 = ce1 * ss1[:, :, None]                          # (S*D, H, 6)
    ce1 = np.ascontiguousarray(
        ce1.reshape(S, D, H, NK).transpose(0, 1, 3, 2)).reshape(S, D * NK, H)
    bf = np.ascontiguousarray(basis.reshape(V, D * NK).astype(np.float32))
    T = np.matmul(bf[None], ce1)                             # (S, V, H)
    T += np.matmul(silu[None], np.asarray(sb1, np.float32).reshape(S, D, H))

    # fold subnode/node affine 1 into T: h = a1*y1 + c1 = sum_s (a1*T + c1/S)
    a1 = (np.asarray(nodes1) * np.asarray(subs1)).astype(np.float32)
    c1 = (np.asarray(nodes1) * np.asarray(subb1) + np.asarray(nodeb1)).astype(np.float32)
    if not (np.all(a1 == 1.0) and np.all(c1 == 0.0)):
        T = a1[None, None, :] * T + (c1 / S)[None, None, :]
    T16 = T.astype(np.float16)

    # layer-2 planes scaled by affine 2: 6 basis (coef2*ss2*a2), silu (sb2*a2),
    # const (c2 against an all-ones feature, carried in partition row 0)
    a2 = (np.asarray(nodes2) * np.asarray(subs2)).astype(np.float32)
    c2 = (np.asarray(nodes2) * np.asarray(subb2) + np.asarray(nodeb2)).astype(np.float32)
    ss2 = np.asarray(ss2, np.float32)
    ce2 = np.asarray(coef2, np.float32)
    if not np.all(ss2 == 1.0):
        ce2 = ce2 * ss2[:, :, None]                          # (H, V, 6)
    ce2 = ce2 * a2[None, :, None]
    sb2a = np.asarray(sb2, np.float32) * a2[None, :]
    wconst = np.zeros((H, V), np.float32)
    wconst[0, :] = c2
    w2p_host = np.concatenate(
        [ce2, sb2a[:, :, None], wconst[:, :, None]], axis=2
    ).astype(np.float16)                                     # (H, V, 8)

    idxT = np.asarray(idx).T.astype(np.float16)              # (S, B)

    in_maps = []
    for c in range(N_CORES):
        sl = slice(c * S_LOC, (c + 1) * S_LOC)
        vsl = slice(c * V_LOC, (c + 1) * V_LOC)
        pk_core = np.empty((128, PCOLS), np.float16)
        pk_core[:, :TCOLS] = T16[sl].transpose(1, 0, 2).reshape(V, S_LOC * H)
        pk_core[:, TCOLS:] = w2p_host[:, vsl, :].transpose(0, 2, 1).reshape(H, NP2 * V_LOC)
        idx_core = np.ascontiguousarray(idxT[sl]).reshape(1, S_LOC * B)
        in_maps.append({"pk": pk_core, "idx16": idx_core})
    return in_maps


_last_results = None


def kernel(**inputs) -> np.ndarray:
    global _last_results, _last_device_wall_ns
    from concourse.bass_utils import run_bass_kernel_spmd
    import os

    nc = _get_nc()
    in_maps = _prepare_inputs(**inputs)
    trace = bool(int(os.environ.get("KAN_TRACE", "0")))
    import time as _t; _t0 = _t.perf_counter()
    res = run_bass_kernel_spmd(nc, in_maps, core_ids=list(range(N_CORES)),
                               trace=trace)
    _last_device_wall_ns = int((_t.perf_counter() - _t0) * 1e9)
    _last_results = res
    logits = np.concatenate(
        [res.results[c]["out"] for c in range(N_CORES)], axis=0)  # (V, B)
    return logits.T.astype(np.float32)
